# revision 59
# baseline (speedup 1.0000x reference)
"""Multi-head attention (B=4, S=2048, D=1024, H=16, Hd=64) on 8 NeuronCores.

Sharding: tensor-parallel over heads. Core c owns heads {2c, 2c+1}, i.e. a
128-column slice of Wq/Wk/Wv and the matching 128-row slice of Wo. Each core
computes a full-shape partial output (its heads' contribution through the out
projection); the host sums the 8 partials (f32) plus the exact bias identities
(softmax rows sum to 1 -> bv@Wo + bo added on host; bk cancels in softmax but
is still applied on-device for free).

v8 structure (vs v2): one CONTINUOUS software pipeline over all 256
(batch, q-slice, k-chunk) steps -- no per-batch drain barriers. The two
pacing engines (PE matmul stream and the ACT exp stream at ~1.07us/step)
stay saturated end to end:

  * attention starts as soon as slice 0 of Q/K/V is projected (~8us) instead
    of after the full batch-0 projection (~35us); weight DMAs are issued in
    consumption order, interleaved with the first x-slice loads.
  * softmax reciprocal on DVE (reciprocal_approx_fast on the PE-broadcast
    row-sums; the custom DVE op mis-executes at base partition 64, so the
    broadcast runs first) -- ACT runs a pure stream of 256 big exps.
  * 2-step emission groups: two steps' scores pairs back-to-back, then the
    two PV pairs (depth 3), then fillers. The PE array reconfigures (and
    LDWEIGHTS pull-ahead breaks) at tile-config switches (~100-250ns each);
    grouping same-config matmuls halves the switches per step.
  * readiness-ordered DVE: every PSUM eviction is emitted one group after
    its matmuls. Engine wait queues are 4-deep strict FIFO -- an eviction
    emitted right after its matmuls waits out the PE queue depth and
    head-of-line-blocks every DVE op behind it (~300us of DVE wait in v5).
  * cost-budgeted fillers (~1.8us PE per group, <=2 accp PSUM slots) keep
    the projection/out-projection work smooth so the exp stream paces;
    the budget rises when the projection backlog is deep. Warm-keeper
    matmuls in the fillers' (128,128) array config stop the HAM clock gate
    from re-throttling the PE when the queue runs dry.

Device algorithm per core (all matmuls bf16, f32 PSUM):
  1. QT/KT = Wc^T x^T + b  -> SBUF [128=d', 8192=s] bf16 (h0 rows 0:64,
     h1 rows 64:128); V -> SBUF [token, d'] chunks with ones columns for the
     softmax row-sum rows (VE layout [128, chunk, 2, 132]).
  2. Per (batch, q-slice of 512): 16 k-chunks of 128: scores^T pair
     (row-tiled concurrent, h0 rows 0:64 / h1 rows 64:128) -> one ACT exp
     (N=1024) -> P^T; O^T accumulated per head via [V_h | ones] lhsT (row-sum
     row rides along). Normalize with DVE reciprocal + PE ones-broadcast ->
     OT bf16.
  3. out_partial = OT^T @ Wo per s-tile -> DRAM bf16.
"""

import os
from contextlib import ExitStack

import numpy as np
import ml_dtypes

import concourse.bass as bass
import concourse.mybir as mybir
import concourse.tile as tile
from concourse import bacc, library_config
from concourse.bass_utils import run_bass_kernel_spmd

B, S, D, H, HD = 4, 2048, 1024, 16, 64
BS = B * S                     # 8192 flattened tokens
NCORES = 8
HPC = H // NCORES              # 2 heads per core
DC = HPC * HD                  # 128-wide weight slice per core

F32 = mybir.dt.float32
BF16 = mybir.dt.bfloat16
EXP = mybir.ActivationFunctionType.Exp

_BUILT = None
LAST_EXEC_NS = None
LAST_RESULTS = None


def _build_program():
    nc = bacc.Bacc("TRN2", target_bir_lowering=False, debug=False,
                   num_devices=NCORES)

    qT_d = nc.dram_tensor("qT", [D, BS], BF16, kind="ExternalInput").ap()
    kT_d = nc.dram_tensor("kT", [D, BS], BF16, kind="ExternalInput").ap()
    vT_d = nc.dram_tensor("vT", [D, BS], BF16, kind="ExternalInput").ap()
    wq_d = nc.dram_tensor("wq", [D, DC], BF16, kind="ExternalInput").ap()
    wk_d = nc.dram_tensor("wk", [D, DC], BF16, kind="ExternalInput").ap()
    wv_d = nc.dram_tensor("wv", [D, DC], BF16, kind="ExternalInput").ap()
    wo_d = nc.dram_tensor("wo", [DC, D], BF16, kind="ExternalInput").ap()
    bq_d = nc.dram_tensor("bq", [DC, 1], F32, kind="ExternalInput").ap()
    bk_d = nc.dram_tensor("bk", [DC, 1], F32, kind="ExternalInput").ap()
    out_d = nc.dram_tensor("out", [BS, D], BF16, kind="ExternalOutput").ap()

    with tile.TileContext(nc) as tc, ExitStack() as ctx:
        const = ctx.enter_context(tc.tile_pool(name="const", bufs=1))
        persist = ctx.enter_context(tc.tile_pool(name="persist", bufs=1))
        stage = ctx.enter_context(tc.tile_pool(name="stage", bufs=3))
        ptpool = ctx.enter_context(tc.tile_pool(name="ptpool", bufs=6))
        npool = ctx.enter_context(tc.tile_pool(name="npool", bufs=2))
        ostage = ctx.enter_context(tc.tile_pool(name="ostage", bufs=4))
        # PSUM: psc 2 slots x 2 banks (scores double-buffer)
        #       acc 2 slots x 1 bank (proj accum / out-proj / bcast)
        #       pop 2 slots x 1 bank (per-head O^T accumulators)
        pscp = ctx.enter_context(tc.tile_pool(name="pscp", bufs=2, space="PSUM"))
        accp = ctx.enter_context(tc.tile_pool(name="accp", bufs=2, space="PSUM"))
        pop = ctx.enter_context(tc.tile_pool(name="pop", bufs=2, space="PSUM"))

        # ---- persistent SBUF state -------------------------------------
        QT = persist.tile([128, BS], BF16)          # [d', s]
        KT = persist.tile([128, BS], BF16)
        OT = persist.tile([128, BS], BF16)
        # V extended, per 128-token chunk (free layout [2, 132], abs width 264):
        #   abs cols 0:64    = V_h0          (h0 lhsT = abs 0:65, rsum row 64)
        #   abs col  64      = ones
        #   abs col  68      = ones          (h1 lhsT = abs 68:196, rsum row 0)
        #   abs cols 132:196 = V_h1          (-> h1 lhsT rows 64:128)
        VE = persist.tile([128, 64, 2, 132], BF16)

        # ---- constants --------------------------------------------------
        wq_sb = const.tile([128, 8, DC], BF16)
        wk_sb = const.tile([128, 8, DC], BF16)
        wv_sb = const.tile([128, 8, DC], BF16)
        wo_sb = const.tile([128, D], BF16)
        bq_sb = const.tile([128, 1], F32)
        bk_sb = const.tile([128, 1], F32)
        ones_sb = const.tile([128, 64], BF16)
        ones32_sb = const.tile([128, 64], F32)
        warm_sb = const.tile([128, 8], F32)
        nc.vector.memset(ones_sb[:], 1.0)
        nc.vector.memset(ones32_sb[:], 1.0)
        nc.vector.memset(warm_sb[:], 0.0)
        # weight DMAs are issued in consumption order, interleaved with the
        # first x-slice loads by the lead-in below (wk before K0's x, etc.)
        nc.sync.dma_start(wk_sb[:], wk_d.rearrange("(c p) d -> p c d", p=128))
        nc.sync.dma_start(bk_sb[:], bk_d)
        nc.vector.memset(VE[:], 0.0)
        nc.vector.memset(VE[:, :, 0, 64:65], 1.0)
        nc.vector.memset(VE[:, :, 0, 68:69], 1.0)
        # preload the exp table set during the DMA-bound lead-in
        nc.scalar.activation(warm_sb[:], warm_sb[:], EXP, scale=1.0)

        # ---- deferred-emission scheduler --------------------------------
        # The engines' wait queues are 4-deep strict FIFO: a DVE evacuation
        # emitted right after its matmuls waits ~a full PE-queue depth and
        # head-of-line-blocks every DVE op behind it (measured ~300us of DVE
        # wait time in v5). So every PSUM evacuation is scheduled one GROUP
        # after its matmuls -- by then the matmuls have executed and the DVE
        # op is ready the moment it issues.
        sched = {}
        cur_group = [0]
        defer_on = [False]

        def at_next(fn, lag=1):
            sched.setdefault(cur_group[0] + lag, []).append(fn)

        def emit_or_defer(fn):
            """Defer a PSUM eviction one group in steady state; emit inline
            during the DMA-bound lead-in (where deferred writes could be
            emitted after their readers)."""
            if defer_on[0]:
                at_next(fn)
            else:
                fn()

        # ---- work-unit emitters -----------------------------------------
        def emit_qk_unit(which, ss):
            """Project one 512-token slice of Q or K (8 accum matmuls);
            bias-add eviction runs next group."""
            srcT, w_sb, b_sb, dstT = (
                (qT_d, wq_sb, bq_sb, QT) if which == "q"
                else (kT_d, wk_sb, bk_sb, KT))
            xt = stage.tile([128, 8, 512], BF16, tag="xT")
            nc.sync.dma_start(
                xt[:],
                srcT.rearrange("(c p) s -> p c s", p=128)[
                    :, :, ss * 512:(ss + 1) * 512],
            )
            ps = accp.tile([128, 512], F32, tag="acc", name="psqk")
            for c in range(8):
                nc.tensor.matmul(ps[:], lhsT=w_sb[:, c], rhs=xt[:, c],
                                 start=(c == 0), stop=(c == 7))
            emit_or_defer(lambda: nc.vector.tensor_scalar_add(
                dstT[:, ss * 512:(ss + 1) * 512], ps[:], b_sb[:]))

        def emit_v_load(ss):
            """DMA one 512-token slice of v; returns the staged tile."""
            vt = stage.tile([128, 8, 512], BF16, tag="xT")
            nc.sync.dma_start(
                vt[:],
                vT_d.rearrange("(c p) s -> p c s", p=128)[
                    :, :, ss * 512:(ss + 1) * 512],
            )
            return vt

        def emit_v_unit(vt, ss, st):
            """Project one 128-token chunk of V into the VE layout;
            eviction copy runs next group."""
            chunk = ss * 4 + st
            ps = accp.tile([128, 512], F32, tag="acc", name="psv")
            for c in range(8):
                nc.tensor.matmul(
                    ps[:, 0:DC],
                    lhsT=vt[:, c, st * 128:(st + 1) * 128],
                    rhs=wv_sb[:, c],
                    start=(c == 0), stop=(c == 7))
            emit_or_defer(lambda: nc.vector.tensor_copy(
                VE[:, chunk, :, 0:64],
                ps[:, 0:DC].rearrange("p (a x) -> p a x", a=2)))

        def emit_outproj_half(st_abs, ns, osb):
            """One matmul of the out-projection for a 128-token tile; the
            PSUM eviction runs next group, the out-DMA after the second
            half's eviction."""
            s0 = st_abs * 128
            ps3 = accp.tile([128, 512], F32, tag="acc", name="ps3")
            nc.tensor.matmul(
                ps3[:],
                lhsT=OT[:, s0:s0 + 128],
                rhs=wo_sb[:, ns * 512:(ns + 1) * 512],
                start=True, stop=True)

            def evac():
                nc.vector.tensor_copy(osb[:, ns * 512:(ns + 1) * 512], ps3[:])
                if ns == 1:
                    nc.sync.dma_start(out_d[s0:s0 + 128, :], osb[:])
            emit_or_defer(evac)

        # ---- attention step emitters ------------------------------------
        po_by_qs = {}
        posb_by_qs = {}
        pts = {}

        def emit_scores_exp(b, qs, chunk):
            q0 = b * S + qs * 512
            k0 = b * S + chunk * 128
            psc = pscp.tile([128, 2, 512], F32, tag="sc", name="psc")
            # row-tiled concurrent pair: h0 rows 0:64, h1 rows 64:128
            nc.tensor.matmul(
                psc[:, 0], lhsT=KT[0:64, k0:k0 + 128],
                rhs=QT[0:64, q0:q0 + 512], start=True, stop=True)
            nc.tensor.matmul(
                psc[:, 1], lhsT=KT[64:128, k0:k0 + 128],
                rhs=QT[64:128, q0:q0 + 512], start=True, stop=True)
            pt = ptpool.tile([128, 2, 512], BF16, tag="pt")
            nc.scalar.activation(
                pt.rearrange("p a x -> p (a x)"),
                psc.rearrange("p a x -> p (a x)"),
                EXP, scale=0.125)
            pts[(b, qs, chunk)] = pt

        def emit_pv(b, qs, chunk):
            if chunk == 0:
                po_by_qs[(b, qs)] = [
                    pop.tile([128, 512], F32, tag="po", name=f"po{h}")
                    for h in range(HPC)]
            po = po_by_qs[(b, qs)]
            pt = pts.pop((b, qs, chunk))
            ve_flat = VE[:, b * 16 + chunk, :, :].rearrange(
                "p a x -> p (a x)")
            first = chunk == 0
            last = chunk == 15
            # h0: rows 0:64 = O^T_h0, row 64 = rowsum_h0
            nc.tensor.matmul(
                po[0][0:65, :], lhsT=ve_flat[:, 0:65], rhs=pt[:, 0],
                start=first, stop=last)
            # h1: row 0 = rowsum_h1 (ones at abs 68), rows 64:128 = O^T_h1
            nc.tensor.matmul(
                po[1][:, :], lhsT=ve_flat[:, 68:196], rhs=pt[:, 1],
                start=first, stop=last)

        def emit_po_evac(b, qs):
            # evacuate the finished accumulators to SBUF (frees both po
            # PSUM slots for the next q-slice's PV); scheduled a group
            # after the final PV matmuls so the copies issue ready
            po = po_by_qs.pop((b, qs))
            posb = [npool.tile([128, 512], F32, tag=f"posb{h}",
                               name=f"posb{h}")
                    for h in range(HPC)]
            nc.vector.tensor_copy(posb[0][0:65, :], po[0][0:65, :])
            nc.vector.tensor_copy(posb[1][:, :], po[1][:, :])
            posb_by_qs[(b, qs)] = posb

        def emit_norm_bcast(b, qs):
            # col-tiled concurrent f32 ones-broadcast of the RAW row-sums
            # (PE); the DVE tail (reciprocal + multiplies) runs next group
            posb = posb_by_qs[(b, qs)]
            bcp = accp.tile([128, 512], F32, tag="acc", name="bcp")
            nc.tensor.matmul(bcp[0:64, :], lhsT=ones32_sb[64:65, :],
                             rhs=posb[0][64:65, :], start=True, stop=True)
            nc.tensor.matmul(bcp[64:128, :], lhsT=ones32_sb[0:1, :],
                             rhs=posb[1][0:1, :], start=True, stop=True)

            def norm_tail():
                # full-tile DVE reciprocal (base partition 0 -- the custom
                # DVE op mis-executes at base partition 64), then two DVE
                # multiplies write the normalized O^T
                q0 = b * S + qs * 512
                pos = posb_by_qs.pop((b, qs))
                rrs = npool.tile([128, 512], F32, tag="rrs")
                nc.vector.reciprocal_approx_fast(rrs[:], bcp[:])
                nc.vector.tensor_mul(
                    OT[0:64, q0:q0 + 512], pos[0][0:64, :], rrs[0:64, :])
                nc.vector.tensor_mul(
                    OT[64:128, q0:q0 + 512], pos[1][64:128, :],
                    rrs[64:128, :])
            at_next(norm_tail)

        # ---- filler scheduling ------------------------------------------
        # Each filler is (pe_cost_ns, accp_slots, fn). The per-group filler
        # budget matches the ACT exp budget (~2.2us/group) minus the
        # attention matmuls (~1.2us), so PE work stays smooth and the exp
        # stream paces the kernel; accp allocations stay <=2 per group
        # (deferred evictions free a slot one group later).
        fillers = []      # urgent: projections (gate the next batch's math)
        fillers2 = []     # deferrable: out-projection halves (only gate DMA)

        def _queue():
            return fillers if fillers else fillers2

        def run_filler():
            """Pop and emit one filler unit (legacy unbudgeted path for the
            lead-in); follow-up units are queued to run next (in order).
            Returns True if a unit ran."""
            q = _queue()
            if not q:
                return False
            _, _, u = q.pop(0)
            r = u()
            if isinstance(r, list):
                q[0:0] = r
            return True

        def run_fillers_budgeted(cost_budget, accp_budget):
            """Pop fillers (urgent queue first) while both budgets last;
            stop at the first unit that does not fit. Returns
            (cost_used, accp_used)."""
            used = 0.0
            accp_used = 0
            while True:
                q = _queue()
                if not q:
                    break
                cost, accs, _ = q[0]
                if used + cost > cost_budget or accp_used + accs > accp_budget:
                    break
                _, _, u = q.pop(0)
                r = u()
                if isinstance(r, list):
                    q[0:0] = r
                used += cost
                accp_used += accs
            return used, accp_used

        def emit_warm_dummy():
            """Tiny matmul that keeps the HAM activity counter fed when the
            filler queue is dry (~27ns of PE, result never read). Uses the
            same (128,128) array config as PV/fillers so it costs no array
            reconfiguration (LDWEIGHTS pull-ahead keeps working around it)."""
            dps = accp.tile([128, 512], F32, tag="acc", name="warmmm")
            nc.tensor.matmul(dps[0:65, 0:64], lhsT=wo_sb[:, 0:65],
                             rhs=ones_sb[:, 0:64], start=True, stop=True)

        def emit_warm_dummy_tail():
            """Tail warm-keeper: after the last scores pair the psc PSUM
            pool is dead, so big write-only matmuls can run from it without
            touching the accp budget. Keeps the clock at 2.4GHz through the
            final norm -> out-projection -> DMA chain (v12 spent ~17us of
            the tail at K=4/8)."""
            dsc = pscp.tile([128, 2, 512], F32, tag="sc", name="warmtail")
            nc.tensor.matmul(dsc[0:65, 0, :], lhsT=wo_sb[:, 0:65],
                             rhs=QT[:, 0:512], start=True, stop=True)
            nc.tensor.matmul(dsc[0:65, 1, :], lhsT=wo_sb[:, 0:65],
                             rhs=QT[:, 512:1024], start=True, stop=True)

        QK_COST, V_COST, OP_COST = 1760.0, 500.0, 250.0

        def proj_units_for_batch(b, k_first):
            """Projection units for batch b as (cost, accp, fn) fillers.

            k_first=True (lead-in residual): all K and V slices first --
            q-slice 0's scores/PV consume them within its first 16 steps --
            then the remaining Q slices.
            k_first=False (steady state): interleave [K, Q, V] per slice."""
            units = []

            def v_group(ss):
                vt = emit_v_load(ss)
                return [(V_COST, 1,
                         lambda st=st, vt=vt, ss=ss: emit_v_unit(vt, ss, st))
                        for st in range(4)]

            if k_first:
                for ss_local in range(1, 4):
                    ss = b * 4 + ss_local
                    units.append((QK_COST, 1,
                                  lambda ss=ss: emit_qk_unit("k", ss)))
                    units.append((0.0, 0, lambda ss=ss: v_group(ss)))
                for ss_local in range(1, 4):
                    ss = b * 4 + ss_local
                    units.append((QK_COST, 1,
                                  lambda ss=ss: emit_qk_unit("q", ss)))
            else:
                # K-first: the next batch's first scores need ALL K slices
                # plus Q slice 0; V chunks follow within its first q-slice.
                # Q slices 1-3 are only due 16/32/48 steps into batch b, so
                # they are scheduled INTO batch b (prepended to the urgent
                # queue ~6 groups before their due step) -- this sheds
                # ~5us of projection matmuls from the PE-overloaded
                # preceding batch into the batch's own slack.
                for ss_local in range(4):
                    ss = b * 4 + ss_local
                    units.append((QK_COST, 1,
                                  lambda ss=ss: emit_qk_unit("k", ss)))
                units.append((QK_COST, 1,
                              lambda ss=b * 4: emit_qk_unit("q", ss)))
                for ss_local in range(4):
                    ss = b * 4 + ss_local
                    units.append((0.0, 0, lambda ss=ss: v_group(ss)))
                for ss_local in range(1, 4):
                    ss = b * 4 + ss_local
                    at_next(lambda ss=ss: fillers.insert(
                        0, (QK_COST, 1,
                            lambda: emit_qk_unit("q", ss))),
                        lag=32 + 8 * ss_local - 6)
            return units

        # ---- lead-in: slice 0 of batch 0 only ---------------------------
        emit_qk_unit("k", 0)
        nc.sync.dma_start(wq_sb[:], wq_d.rearrange("(c p) d -> p c d", p=128))
        nc.sync.dma_start(bq_sb[:], bq_d)
        emit_qk_unit("q", 0)
        nc.sync.dma_start(wv_sb[:], wv_d.rearrange("(c p) d -> p c d", p=128))
        lead_v = emit_v_load(0)
        nc.sync.dma_start(wo_sb[:], wo_d)
        for st in range(4):
            emit_v_unit(lead_v, 0, st)
        fillers.extend(proj_units_for_batch(0, True))

        # ---- main pipeline: one continuous 256-step loop, 2-step groups --
        # The PE array reconfigures (and LDWEIGHTS pull-ahead breaks) when
        # the tile config changes, exposing ~100-250ns per switch. The
        # scores pairs use a (64,128) row-tiled config; everything else is
        # (128,128). Grouping two steps' scores back-to-back, then the two
        # matching PV pairs (depth 3), then fillers, halves the number of
        # config switches per step. The psc double-buffer allows exactly a
        # 2-step scores run-ahead. Each group: scheduled (ready) DVE work
        # first, then scores pairs, PV pairs, norm broadcast, PE fillers.
        NSTEP = B * 64
        NGROUP = NSTEP // 2 + 12
        for gidx in range(NGROUP):
            cur_group[0] = gidx
            defer_on[0] = gidx >= 8
            g2 = gidx * 2
            for fn in sched.pop(gidx, ()):
                fn()
            for gi in (g2, g2 + 1):
                if gi < NSTEP:
                    b, r = divmod(gi, 64)
                    qs, chunk = divmod(r, 16)
                    emit_scores_exp(b, qs, chunk)
                    if r == 0 and b + 1 < B:
                        # queue next batch's projections at batch start
                        fillers.extend(proj_units_for_batch(b + 1, False))
            for gi in (g2, g2 + 1):
                d = gi - 3
                if 0 <= d < NSTEP:
                    db, dr = divmod(d, 64)
                    dq, dc = divmod(dr, 16)
                    emit_pv(db, dq, dc)
                    if dc == 15:
                        at_next(lambda db=db, dq=dq: emit_po_evac(db, dq))
            normed = False
            for gi in (g2, g2 + 1):
                e = gi - 8
                if 0 <= e < NSTEP and e % 16 == 15:
                    eb, er = divmod(e, 64)
                    eq = er // 16
                    emit_norm_bcast(eb, eq)
                    normed = True
                    base = (eb * S + eq * 512) // 128

                    # out-projection sub-units (one matmul each) as fillers
                    # two groups later, once OT is written
                    def queue_outproj(base=base):
                        for k in range(4):
                            osb = ostage.tile([128, D], BF16, tag="osb")
                            fillers2.append(
                                (OP_COST, 1,
                                 lambda st=base + k, osb=osb:
                                 emit_outproj_half(st, 0, osb)))
                            fillers2.append(
                                (OP_COST, 1,
                                 lambda st=base + k, osb=osb:
                                 emit_outproj_half(st, 1, osb)))
                    at_next(queue_outproj, lag=2)
            # fillers: budget the group's filler cost so PE work stays
            # smooth (ACT paces), and cap accp allocations at 2 per group
            # (1 on norm groups -- bcp holds a slot).  Lead-in groups run
            # unbudgeted with IMMEDIATE evictions (DMA-bound anyway).
            if g2 < 16:
                for _ in range(4):
                    run_filler()
            else:
                # dynamic budget: drain faster when the projection backlog
                # is deep (batch starts), so the next batch's Q/K/V are
                # ready before its scores come due
                backlog = sum(c for c, _, _ in fillers) + \
                    sum(c for c, _, _ in fillers2)
                budget = 2400.0 if backlog > 8000.0 else 1800.0
                used, accs = run_fillers_budgeted(
                    budget, 1 if normed else 2)
                if used < 250.0 and accs < 2 and g2 < NSTEP and not normed:
                    emit_warm_dummy()
                elif g2 >= NSTEP:
                    emit_warm_dummy_tail()

        # drain any unfinished scheduled work and fillers (at_next during
        # the drain may add further entries); fillers stay rate-limited to
        # two per virtual group so deferred accp evictions keep pace
        gidx = NGROUP
        while sched or fillers or fillers2:
            cur_group[0] = gidx
            for fn in sched.pop(gidx, ()):
                fn()
            for _ in range(2):
                if fillers or fillers2:
                    run_filler()
            gidx += 1

    nc.compile()
    return nc


def _get_program():
    global _BUILT
    if _BUILT is None:
        _BUILT = _build_program()
    return _BUILT


def kernel(q, k, v, Wq, bq, Wk, bk, Wv, bv, Wo, bo, trace=None):
    global LAST_EXEC_NS, LAST_RESULTS
    if trace is None:
        trace = os.environ.get("KERNEL_TRACE", "0") == "1"
    bf16 = ml_dtypes.bfloat16

    q2 = np.asarray(q, np.float32).reshape(BS, D)
    k2 = np.asarray(k, np.float32).reshape(BS, D)
    v2 = np.asarray(v, np.float32).reshape(BS, D)
    qT = np.ascontiguousarray(q2.T).astype(bf16)
    kT = np.ascontiguousarray(k2.T).astype(bf16)
    vT = np.ascontiguousarray(v2.T).astype(bf16)

    Wq = np.asarray(Wq, np.float32)
    Wk = np.asarray(Wk, np.float32)
    Wv = np.asarray(Wv, np.float32)
    Wo = np.asarray(Wo, np.float32)
    bq = np.asarray(bq, np.float32)
    bk = np.asarray(bk, np.float32)
    bv = np.asarray(bv, np.float32)
    bo = np.asarray(bo, np.float32)

    in_maps = []
    for c in range(NCORES):
        sl = slice(c * DC, (c + 1) * DC)
        in_maps.append({
            "qT": qT, "kT": kT, "vT": vT,
            "wq": np.ascontiguousarray(Wq[:, sl]).astype(bf16),
            "wk": np.ascontiguousarray(Wk[:, sl]).astype(bf16),
            "wv": np.ascontiguousarray(Wv[:, sl]).astype(bf16),
            "wo": np.ascontiguousarray(Wo[sl, :]).astype(bf16),
            "bq": np.ascontiguousarray(bq[sl]).reshape(DC, 1),
            "bk": np.ascontiguousarray(bk[sl]).reshape(DC, 1),
        })

    nc = _get_program()
    res = run_bass_kernel_spmd(nc, in_maps, list(range(NCORES)), trace=trace)
    LAST_EXEC_NS = res.exec_time_ns
    LAST_RESULTS = res

    out = np.zeros((BS, D), np.float32)
    for c in range(NCORES):
        out += np.asarray(res.results[c]["out"], np.float32)
    out += bv.astype(np.float32) @ Wo + bo          # exact bias identities
    return out.reshape(B, S, D)
